# revision 14
# baseline (speedup 1.0000x reference)
"""Multi-head attention (B=4, N=1024, C=1024, H=16) on 8 TRN2 NeuronCores.

Sharding: batch B (4-way) x head-group (2-way, 8 heads each).
Core i handles batch b=i//2, head-group hg=i%2.

Per-core outputs: attn_p [Nq,Nk] (sum of the group's 8 scaled score maps)
and y_p [N,C] (projection partial over the group's 512 channels).
Host unshard: pair-sum + bias + /16 head-mean + final transposes.
"""
import numpy as np
import ml_dtypes

import concourse.mybir as mybir
import concourse.tile as tile
from concourse import bacc
from concourse.bass_utils import run_bass_kernel_spmd

P = 128
B, N, C, H = 4, 1024, 1024, 16
HG = 2            # head-group shards
CH = C // HG      # 512 channels per core
NHEAD = H // HG   # 8 heads per core
D = C // H        # 64 head dim
KC = C // P       # 8 contraction chunks over C
CC = CH // P      # 4 chunks over the 512 sharded channels
QT = N // P       # 8 token tiles of 128
QH = N // 512     # 2 token halves of 512
BF = mybir.dt.bfloat16
F32 = mybir.dt.float32

_NC_CACHE = None


def _build_nc(loop_n=1):
    nc = bacc.Bacc("TRN2", target_bir_lowering=False, debug=False, num_devices=8)

    xqT = nc.dram_tensor("xqT", [C, N], BF, kind="ExternalInput")
    xkT = nc.dram_tensor("xkT", [C, N], BF, kind="ExternalInput")
    xvT = nc.dram_tensor("xvT", [C, N], BF, kind="ExternalInput")
    wqT = nc.dram_tensor("wqT", [C, CH], BF, kind="ExternalInput")
    wkT = nc.dram_tensor("wkT", [C, CH], BF, kind="ExternalInput")
    wvT = nc.dram_tensor("wvT", [C, CH], BF, kind="ExternalInput")
    wpT = nc.dram_tensor("wpT", [CH, C], BF, kind="ExternalInput")
    attn_p = nc.dram_tensor("attn_p", [N, N], F32, kind="ExternalOutput")
    y_p = nc.dram_tensor("y_p", [N, C], F32, kind="ExternalOutput")

    xqT3 = xqT.rearrange("(ko p) n -> p ko n", p=P)
    xkT3 = xkT.rearrange("(ko p) n -> p ko n", p=P)
    xvT3 = xvT.rearrange("(ko p) n -> p ko n", p=P)
    wqT3 = wqT.rearrange("(ko p) m -> p ko m", p=P)
    wkT3 = wkT.rearrange("(ko p) m -> p ko m", p=P)
    wvT3 = wvT.rearrange("(ko p) m -> p ko m", p=P)
    wpT3 = wpT.rearrange("(ko p) m -> p ko m", p=P)
    attn3 = attn_p.rearrange("(qo p) k -> p qo k", p=P)
    y3 = y_p.rearrange("(to p) c -> p to c", p=P)

    with tile.TileContext(nc) as tc:
        with (
            tc.tile_pool(name="inp", bufs=1) as inp,
            tc.tile_pool(name="w", bufs=1) as wp_,
            tc.tile_pool(name="qkv", bufs=1) as qkv,
            tc.tile_pool(name="pt", bufs=5) as ptp,
            tc.tile_pool(name="sm", bufs=2) as smp,
            tc.tile_pool(name="psA", bufs=3, space="PSUM") as psA,
            tc.tile_pool(name="psS", bufs=3, space="PSUM") as psS,
            tc.tile_pool(name="psO", bufs=2, space="PSUM") as psO,
        ):
            if loop_n > 1:
                loop_ctx = tc.For_i(0, loop_n, 1)
                loop_ctx.__enter__()
            # ---- input / weight loads (chunked for DMA-queue parallelism) ----
            xq_sb = inp.tile([P, KC, N], BF, tag="xq")
            xk_sb = inp.tile([P, KC, N], BF, tag="xk")
            xv_sb = inp.tile([P, KC, N], BF, tag="xv")
            wq_sb = wp_.tile([P, KC, CH], BF, tag="wq")
            wk_sb = wp_.tile([P, KC, CH], BF, tag="wk")
            wv_sb = wp_.tile([P, KC, CH], BF, tag="wv")
            wpp_sb = wp_.tile([P, CC, C], BF, tag="wp")
            for kc in range(KC):
                nc.sync.dma_start(xq_sb[:, kc, :], xqT3[:, kc, :])
                nc.sync.dma_start(wq_sb[:, kc, :], wqT3[:, kc, :])
            for kc in range(KC):
                nc.sync.dma_start(xk_sb[:, kc, :], xkT3[:, kc, :])
                nc.sync.dma_start(wk_sb[:, kc, :], wkT3[:, kc, :])
            for kc in range(KC):
                nc.sync.dma_start(xv_sb[:, kc, :], xvT3[:, kc, :])
                nc.sync.dma_start(wv_sb[:, kc, :], wvT3[:, kc, :])
            for cc in range(CC):
                nc.sync.dma_start(wpp_sb[:, cc, :], wpT3[:, cc, :])

            qt_sb = qkv.tile([P, CC, N], BF, tag="qt")    # Q^T (pre-scaled by 1/8)
            kt_sb = qkv.tile([P, CC, N], BF, tag="kt")    # K^T
            v65_sb = qkv.tile([P, QT, NHEAD * (D + 1)], BF, tag="v65")
            ot_sb = qkv.tile([P, CC, N], BF, tag="ot")    # O^T (normalized)

            def gen_qk(cc):
                """Q^T and K^T chunk cc: 32 matmul quanta."""
                for w_sb, x_sb, o_sb, scale in (
                    (wq_sb, xq_sb, qt_sb, 0.125),
                    (wk_sb, xk_sb, kt_sb, None),
                ):
                    pss = [psA.tile([P, 512], F32, tag="pa", name=f"psqk{qh}")
                           for qh in range(QH)]
                    for kc in range(KC):
                        for qh in range(QH):
                            nc.tensor.matmul(
                                pss[qh][:],
                                lhsT=w_sb[:, kc, cc * P:(cc + 1) * P],
                                rhs=x_sb[:, kc, qh * 512:(qh + 1) * 512],
                                start=(kc == 0),
                                stop=(kc == KC - 1),
                            )
                            yield
                    for qh in range(QH):
                        dst = o_sb[:, cc, qh * 512:(qh + 1) * 512]
                        if scale is not None:
                            nc.vector.tensor_scalar_mul(dst, pss[qh][:], scale)
                        else:
                            nc.vector.tensor_copy(out=dst, in_=pss[qh][:])
                yield f"qk{cc}"

            def gen_v():
                """V projection into [tok, head*(64+1)] layout: 64 quanta."""
                for tt in range(QT):
                    pv = psA.tile([P, 512], F32, tag="pa", name="pv")
                    for kc in range(KC):
                        nc.tensor.matmul(
                            pv[:],
                            lhsT=xv_sb[:, kc, tt * P:(tt + 1) * P],
                            rhs=wv_sb[:, kc, :],
                            start=(kc == 0),
                            stop=(kc == KC - 1),
                        )
                        yield
                    for h in range(NHEAD):
                        nc.vector.tensor_copy(
                            out=v65_sb[:, tt, h * (D + 1): h * (D + 1) + D],
                            in_=pv[:, h * D:(h + 1) * D],
                        )
                    nc.any.memset(v65_sb[:, tt, D::(D + 1)], 1.0)

            pts_by_pair = {}

            def gen_scores(hp):
                """16 slots; each: 2 row-packed matmuls (concurrent) + 2 exps."""
                pts = [ptp.tile([P, QT, N], BF, tag="pt", name=f"pt{h2}")
                       for h2 in range(2)]
                pts_by_pair[hp] = pts
                for kt in range(QT):
                    for qh in range(QH):
                        pss = []
                        for h2 in range(2):
                            base = 64 * h2
                            ps = psS.tile([P, 512], F32, tag="ps", name="ps")
                            nc.tensor.matmul(
                                ps[:],
                                lhsT=kt_sb[base:base + 64, hp, kt * P:(kt + 1) * P],
                                rhs=qt_sb[base:base + 64, hp, qh * 512:(qh + 1) * 512],
                                start=True,
                                stop=True,
                            )
                            pss.append(ps)
                        for h2 in range(2):
                            nc.scalar.activation(
                                pts[h2][:, kt, qh * 512:(qh + 1) * 512], pss[h2][:],
                                mybir.ActivationFunctionType.Exp,
                            )
                        yield

            def gen_pv(hp):
                """PV + normalize for head pair hp: 32 quanta."""
                pts = pts_by_pair[hp]
                for h2 in range(2):
                    h = 2 * hp + h2
                    base = 64 * h2
                    pos = [psO.tile([65, 512], F32, tag="po", name=f"po{qh}")
                           for qh in range(QH)]
                    for kt in range(QT):
                        for qh in range(QH):
                            nc.tensor.matmul(
                                pos[qh][:],
                                lhsT=v65_sb[:, kt, h * (D + 1):(h + 1) * (D + 1)],
                                rhs=pts[h2][:, kt, qh * 512:(qh + 1) * 512],
                                start=(kt == 0),
                                stop=(kt == QT - 1),
                            )
                            yield
                    for qh in range(QH):
                        recip = smp.tile([1, 512], F32, tag="recip", name="recip")
                        nc.vector.reciprocal(recip[:], pos[qh][64:65, :])
                        bc = smp.tile([64, 512], F32, tag="bc", name="bc")
                        nc.gpsimd.partition_broadcast(bc[:], recip[:])
                        nc.vector.tensor_mul(
                            out=ot_sb[base:base + 64, hp, qh * 512:(qh + 1) * 512],
                            in0=pos[qh][0:64, :],
                            in1=bc[:],
                        )

            def gen_b():
                """attn partial (full 512-ch contraction): 64 quanta."""
                for qt in range(QT):
                    pas = [psA.tile([P, 512], F32, tag="pa", name=f"pat{kh}")
                           for kh in range(QH)]
                    for cc in range(CC):
                        for kh in range(QH):
                            nc.tensor.matmul(
                                pas[kh][:],
                                lhsT=qt_sb[:, cc, qt * P:(qt + 1) * P],
                                rhs=kt_sb[:, cc, kh * 512:(kh + 1) * 512],
                                start=(cc == 0),
                                stop=(cc == CC - 1),
                            )
                            yield
                    for kh in range(QH):
                        stg = smp.tile([P, 512], F32, tag="stg", name="stga")
                        nc.vector.tensor_copy(out=stg[:], in_=pas[kh][:])
                        nc.sync.dma_start(attn3[:, qt, kh * 512:(kh + 1) * 512], stg[:])

            def gen_d():
                """output projection y partial [N, C]: 64 quanta."""
                for qt in range(QT):
                    pys = [psA.tile([P, 512], F32, tag="pa", name=f"py{ch2}")
                           for ch2 in range(QH)]
                    for cc in range(CC):
                        for ch2 in range(QH):
                            nc.tensor.matmul(
                                pys[ch2][:],
                                lhsT=ot_sb[:, cc, qt * P:(qt + 1) * P],
                                rhs=wpp_sb[:, cc, ch2 * 512:(ch2 + 1) * 512],
                                start=(cc == 0),
                                stop=(cc == CC - 1),
                            )
                            yield
                    for ch2 in range(QH):
                        stg = smp.tile([P, 512], F32, tag="stg", name="stgy")
                        nc.vector.tensor_copy(out=stg[:], in_=pys[ch2][:])
                        nc.sync.dma_start(y3[:, qt, ch2 * 512:(ch2 + 1) * 512], stg[:])

            # interleaved emission: qk0 first, then 1 score slot : 4 other MMs.
            # scores for pair hp are gated on the "qk{hp}" marker so every
            # kt/qt evacuation is emitted before its first reader.
            import itertools
            for _ in gen_qk(0):
                pass
            others = itertools.chain(
                gen_qk(1), gen_v(), gen_qk(2), gen_qk(3), gen_pv(0),
                gen_pv(1), gen_b(), gen_pv(2), gen_pv(3), gen_d(),
            )
            _SENT = object()
            events = {"qk0"}

            def pump():
                x = next(others, _SENT)
                if x is _SENT:
                    return False
                if isinstance(x, str):
                    events.add(x)
                return True

            for hp in range(NHEAD // 2):
                while f"qk{hp}" not in events:
                    if not pump():
                        break
                for _slot in gen_scores(hp):
                    for _i in range(4):
                        pump()
            while pump():
                pass

            if loop_n > 1:
                loop_ctx.__exit__(None, None, None)

    nc.compile()
    return nc


def get_nc():
    global _NC_CACHE
    if _NC_CACHE is None:
        _NC_CACHE = _build_nc()
    return _NC_CACHE


def _prep_in_maps(xq, xk, xv, Wq, Wk, Wv, Wp):
    bf = ml_dtypes.bfloat16
    in_maps = []
    xqT = [np.ascontiguousarray(xq[b].T).astype(bf) for b in range(B)]
    xkT = [np.ascontiguousarray(xk[b].T).astype(bf) for b in range(B)]
    xvT = [np.ascontiguousarray(xv[b].T).astype(bf) for b in range(B)]
    wqTs = [np.ascontiguousarray(Wq[hg * CH:(hg + 1) * CH, :].T).astype(bf) for hg in range(HG)]
    wkTs = [np.ascontiguousarray(Wk[hg * CH:(hg + 1) * CH, :].T).astype(bf) for hg in range(HG)]
    wvTs = [np.ascontiguousarray(Wv[hg * CH:(hg + 1) * CH, :].T).astype(bf) for hg in range(HG)]
    wpTs = [np.ascontiguousarray(Wp[:, hg * CH:(hg + 1) * CH].T).astype(bf) for hg in range(HG)]
    for core in range(8):
        b, hg = core // HG, core % HG
        in_maps.append({
            "xqT": xqT[b], "xkT": xkT[b], "xvT": xvT[b],
            "wqT": wqTs[hg], "wkT": wkTs[hg], "wvT": wvTs[hg],
            "wpT": wpTs[hg],
        })
    return in_maps


def kernel(xq, xk, xv, Wq, Wk, Wv, Wp, bp):
    xq, xk, xv = (np.asarray(t, np.float32) for t in (xq, xk, xv))
    Wq, Wk, Wv, Wp, bp = (np.asarray(t, np.float32) for t in (Wq, Wk, Wv, Wp, bp))
    nc = get_nc()
    in_maps = _prep_in_maps(xq, xk, xv, Wq, Wk, Wv, Wp)
    res = run_bass_kernel_spmd(nc, in_maps, list(range(8)))

    x_out = np.empty((B, N, C), np.float32)
    attn = np.empty((B, N, N), np.float32)
    for b in range(B):
        r0, r1 = res.results[2 * b], res.results[2 * b + 1]
        attn[b] = (r0["attn_p"] + r1["attn_p"]) * (1.0 / H)
        x_out[b] = r0["y_p"] + r1["y_p"] + bp
    return x_out.swapaxes(0, 1), attn


# revision 17
# speedup vs baseline: 1.0610x; 1.0610x over previous
"""Multi-head attention (B=4, N=1024, C=1024, H=16) on 8 TRN2 NeuronCores.

Sharding: batch B (4-way) x head-group (2-way, 8 heads each).
Core i handles batch b=i//2, head-group hg=i%2.

Per-core outputs: attn_p [Nq,Nk] (sum of the group's 8 scaled score maps)
and y_p [N,C] (projection partial over the group's 512 channels).
Host unshard: pair-sum + bias + /16 head-mean + final transposes.
"""
import numpy as np
import ml_dtypes

import concourse.mybir as mybir
import concourse.tile as tile
from concourse import bacc
from concourse.bass_utils import run_bass_kernel_spmd

P = 128
B, N, C, H = 4, 1024, 1024, 16
HG = 2            # head-group shards
CH = C // HG      # 512 channels per core
NHEAD = H // HG   # 8 heads per core
D = C // H        # 64 head dim
KC = C // P       # 8 contraction chunks over C
CC = CH // P      # 4 chunks over the 512 sharded channels
QT = N // P       # 8 token tiles of 128
QH = N // 512     # 2 token halves of 512
BF = mybir.dt.bfloat16
F32 = mybir.dt.float32

_NC_CACHE = None


def _build_nc(loop_n=1, variant=""):
    vset = set(variant.split(",")) if variant else set()
    nc = bacc.Bacc("TRN2", target_bir_lowering=False, debug=False, num_devices=8)

    xqT = nc.dram_tensor("xqT", [C, N], BF, kind="ExternalInput")
    xkT = nc.dram_tensor("xkT", [C, N], BF, kind="ExternalInput")
    xvT = nc.dram_tensor("xvT", [C, N], BF, kind="ExternalInput")
    wqT = nc.dram_tensor("wqT", [C, CH], BF, kind="ExternalInput")
    wkT = nc.dram_tensor("wkT", [C, CH], BF, kind="ExternalInput")
    wvT = nc.dram_tensor("wvT", [C, CH], BF, kind="ExternalInput")
    wpT = nc.dram_tensor("wpT", [CH, C], BF, kind="ExternalInput")
    attn_p = nc.dram_tensor("attn_p", [N, N], F32, kind="ExternalOutput")
    y_p = nc.dram_tensor("y_p", [N, C], F32, kind="ExternalOutput")

    xqT3 = xqT.rearrange("(ko p) n -> p ko n", p=P)
    xkT3 = xkT.rearrange("(ko p) n -> p ko n", p=P)
    xvT3 = xvT.rearrange("(ko p) n -> p ko n", p=P)
    wqT3 = wqT.rearrange("(ko p) m -> p ko m", p=P)
    wkT3 = wkT.rearrange("(ko p) m -> p ko m", p=P)
    wvT3 = wvT.rearrange("(ko p) m -> p ko m", p=P)
    wpT3 = wpT.rearrange("(ko p) m -> p ko m", p=P)
    attn3 = attn_p.rearrange("(qo p) k -> p qo k", p=P)
    y3 = y_p.rearrange("(to p) c -> p to c", p=P)

    with tile.TileContext(nc) as tc:
        with (
            tc.tile_pool(name="inp", bufs=1) as inp,
            tc.tile_pool(name="w", bufs=1) as wp_,
            tc.tile_pool(name="qkv", bufs=1) as qkv,
            tc.tile_pool(name="pt", bufs=5) as ptp,
            tc.tile_pool(name="sm", bufs=2) as smp,
            tc.tile_pool(name="st", bufs=3) as stp,
            tc.tile_pool(name="psA", bufs=3, space="PSUM") as psA,
            tc.tile_pool(name="psS", bufs=2, space="PSUM") as psS,
            tc.tile_pool(name="psO", bufs=3, space="PSUM") as psO,
        ):
            if loop_n > 1 and "dma_out" not in vset:
                loop_ctx = tc.For_i(0, loop_n, 1)
                loop_ctx.__enter__()
            # ---- input / weight loads (chunked for DMA-queue parallelism) ----
            xq_sb = inp.tile([P, KC, N], BF, tag="xq")
            xk_sb = inp.tile([P, KC, N], BF, tag="xk")
            xv_sb = inp.tile([P, KC, N], BF, tag="xv")
            wq_sb = wp_.tile([P, KC, CH], BF, tag="wq")
            wk_sb = wp_.tile([P, KC, CH], BF, tag="wk")
            wv_sb = wp_.tile([P, KC, CH], BF, tag="wv")
            wpp_sb = wp_.tile([P, CC, C], BF, tag="wp")
            for kc in range(KC):
                nc.sync.dma_start(xq_sb[:, kc, :], xqT3[:, kc, :])
                nc.sync.dma_start(wq_sb[:, kc, :], wqT3[:, kc, :])
            for kc in range(KC):
                nc.sync.dma_start(xk_sb[:, kc, :], xkT3[:, kc, :])
                nc.sync.dma_start(wk_sb[:, kc, :], wkT3[:, kc, :])
            for kc in range(KC):
                nc.sync.dma_start(xv_sb[:, kc, :], xvT3[:, kc, :])
                nc.sync.dma_start(wv_sb[:, kc, :], wvT3[:, kc, :])
            for cc in range(CC):
                nc.sync.dma_start(wpp_sb[:, cc, :], wpT3[:, cc, :])

            if loop_n > 1 and "dma_out" in vset:
                loop_ctx = tc.For_i(0, loop_n, 1)
                loop_ctx.__enter__()
            qt_sb = qkv.tile([P, CC, N], BF, tag="qt")    # Q^T (pre-scaled by 1/8)
            kt_sb = qkv.tile([P, CC, N], BF, tag="kt")    # K^T
            v65_sb = qkv.tile([P, QT, NHEAD * (D + 1)], BF, tag="v65")
            ot_sb = qkv.tile([P, CC, N], BF, tag="ot")    # O^T (normalized)

            def gen_qk(cc):
                """Q^T and K^T chunk cc: 32 matmul quanta."""
                for w_sb, x_sb, o_sb, scale in (
                    (wq_sb, xq_sb, qt_sb, 0.125),
                    (wk_sb, xk_sb, kt_sb, None),
                ):
                    pss = [psA.tile([P, 512], F32, tag="pa", name=f"psqk{qh}")
                           for qh in range(QH)]
                    for kc in range(KC):
                        for qh in range(QH):
                            nc.tensor.matmul(
                                pss[qh][:],
                                lhsT=w_sb[:, kc, cc * P:(cc + 1) * P],
                                rhs=x_sb[:, kc, qh * 512:(qh + 1) * 512],
                                start=(kc == 0),
                                stop=(kc == KC - 1),
                            )
                            yield
                    for qh in range(QH):
                        dst = o_sb[:, cc, qh * 512:(qh + 1) * 512]
                        if scale is not None:
                            nc.vector.tensor_scalar_mul(dst, pss[qh][:], scale)
                        else:
                            nc.vector.tensor_copy(out=dst, in_=pss[qh][:])
                yield f"qk{cc}"

            def gen_v():
                """V projection into [tok, head*(64+1)] layout: 64 quanta."""
                for tt in range(QT):
                    pv = psA.tile([P, 512], F32, tag="pa", name="pv")
                    for kc in range(KC):
                        nc.tensor.matmul(
                            pv[:],
                            lhsT=xv_sb[:, kc, tt * P:(tt + 1) * P],
                            rhs=wv_sb[:, kc, :],
                            start=(kc == 0),
                            stop=(kc == KC - 1),
                        )
                        yield
                    for h in range(NHEAD):
                        nc.vector.tensor_copy(
                            out=v65_sb[:, tt, h * (D + 1): h * (D + 1) + D],
                            in_=pv[:, h * D:(h + 1) * D],
                        )
                    nc.any.memset(v65_sb[:, tt, D::(D + 1)], 1.0)

            pts_by_pair = {}

            def gen_scores(hp):
                """16 slots; each: 2 row-packed matmuls (concurrent) + 2 exps."""
                pts = [ptp.tile([P, QT, N], BF, tag="pt", name=f"pt{h2}")
                       for h2 in range(2)]
                pts_by_pair[hp] = pts
                for kt in range(QT):
                    for qh in range(QH):
                        pss = []
                        for h2 in range(2):
                            base = 64 * h2
                            ps = psS.tile([P, 512], F32, tag="ps", name="ps")
                            nc.tensor.matmul(
                                ps[:],
                                lhsT=kt_sb[base:base + 64, hp, kt * P:(kt + 1) * P],
                                rhs=qt_sb[base:base + 64, hp, qh * 512:(qh + 1) * 512],
                                start=True,
                                stop=True,
                            )
                            pss.append(ps)
                        if "noexp" not in vset:
                            for h2 in range(2):
                                nc.scalar.activation(
                                    pts[h2][:, kt, qh * 512:(qh + 1) * 512], pss[h2][:],
                                    mybir.ActivationFunctionType.Exp,
                                )
                        yield

            def gen_pv(hp):
                """PV + normalize for head pair hp: 32 quanta."""
                pts = pts_by_pair[hp]
                for h2 in range(2):
                    h = 2 * hp + h2
                    base = 64 * h2
                    pos = [psO.tile([65, 512], F32, tag="po", name=f"po{qh}")
                           for qh in range(QH)]
                    for kt in range(QT):
                        for qh in range(QH):
                            nc.tensor.matmul(
                                pos[qh][:],
                                lhsT=v65_sb[:, kt, h * (D + 1):(h + 1) * (D + 1)],
                                rhs=pts[h2][:, kt, qh * 512:(qh + 1) * 512],
                                start=(kt == 0),
                                stop=(kt == QT - 1),
                            )
                            yield
                    for qh in range(QH if "nonorm" not in vset else 0):
                        recip = smp.tile([1, 512], F32, tag="recip", name="recip")
                        nc.vector.reciprocal(recip[:], pos[qh][64:65, :])
                        bc = smp.tile([64, 512], F32, tag="bc", name="bc")
                        nc.gpsimd.partition_broadcast(bc[:], recip[:])
                        nc.vector.tensor_mul(
                            out=ot_sb[base:base + 64, hp, qh * 512:(qh + 1) * 512],
                            in0=pos[qh][0:64, :],
                            in1=bc[:],
                        )

            def gen_b():
                """attn partial (full 512-ch contraction): 64 quanta."""
                for qt in range(QT):
                    pas = [psA.tile([P, 512], F32, tag="pa", name=f"pat{kh}")
                           for kh in range(QH)]
                    for cc in range(CC):
                        for kh in range(QH):
                            nc.tensor.matmul(
                                pas[kh][:],
                                lhsT=qt_sb[:, cc, qt * P:(qt + 1) * P],
                                rhs=kt_sb[:, cc, kh * 512:(kh + 1) * 512],
                                start=(cc == 0),
                                stop=(cc == CC - 1),
                            )
                            yield
                    for kh in range(QH):
                        stg = stp.tile([P, 512], F32, tag="stg", name="stga")
                        nc.vector.tensor_copy(out=stg[:], in_=pas[kh][:])
                        nc.sync.dma_start(attn3[:, qt, kh * 512:(kh + 1) * 512], stg[:])

            def gen_d():
                """output projection y partial [N, C]: 64 quanta."""
                for qt in range(QT):
                    pys = [psA.tile([P, 512], F32, tag="pa", name=f"py{ch2}")
                           for ch2 in range(QH)]
                    for cc in range(CC):
                        for ch2 in range(QH):
                            nc.tensor.matmul(
                                pys[ch2][:],
                                lhsT=ot_sb[:, cc, qt * P:(qt + 1) * P],
                                rhs=wpp_sb[:, cc, ch2 * 512:(ch2 + 1) * 512],
                                start=(cc == 0),
                                stop=(cc == CC - 1),
                            )
                            yield
                    for ch2 in range(QH):
                        stg = stp.tile([P, 512], F32, tag="stg", name="stgy")
                        nc.vector.tensor_copy(out=stg[:], in_=pys[ch2][:])
                        nc.sync.dma_start(y3[:, qt, ch2 * 512:(ch2 + 1) * 512], stg[:])

            # interleaved emission: qk0 first, then 1 score slot : 4 other MMs.
            # scores for pair hp are gated on the "qk{hp}" marker so every
            # kt/qt evacuation is emitted before its first reader.
            import itertools
            for _ in gen_qk(0):
                pass
            _chain = [gen_qk(1), gen_v(), gen_qk(2), gen_qk(3)]
            if "nopv" not in vset:
                _chain += [gen_pv(0), gen_pv(1)]
            if "nob" not in vset:
                _chain.append(gen_b())
            if "nopv" not in vset:
                _chain += [gen_pv(2), gen_pv(3)]
            if "nod" not in vset:
                _chain.append(gen_d())
            others = itertools.chain(*_chain)
            _SENT = object()
            events = {"qk0"}

            def pump():
                x = next(others, _SENT)
                if x is _SENT:
                    return False
                if isinstance(x, str):
                    events.add(x)
                return True

            for hp in range(NHEAD // 2):
                while f"qk{hp}" not in events:
                    if not pump():
                        break
                for _slot in gen_scores(hp):
                    for _i in range(4):
                        pump()
            while pump():
                pass

            if loop_n > 1:
                loop_ctx.__exit__(None, None, None)

    nc.compile()
    return nc


def get_nc():
    global _NC_CACHE
    if _NC_CACHE is None:
        _NC_CACHE = _build_nc()
    return _NC_CACHE


def _prep_in_maps(xq, xk, xv, Wq, Wk, Wv, Wp):
    bf = ml_dtypes.bfloat16
    in_maps = []
    xqT = [np.ascontiguousarray(xq[b].T).astype(bf) for b in range(B)]
    xkT = [np.ascontiguousarray(xk[b].T).astype(bf) for b in range(B)]
    xvT = [np.ascontiguousarray(xv[b].T).astype(bf) for b in range(B)]
    wqTs = [np.ascontiguousarray(Wq[hg * CH:(hg + 1) * CH, :].T).astype(bf) for hg in range(HG)]
    wkTs = [np.ascontiguousarray(Wk[hg * CH:(hg + 1) * CH, :].T).astype(bf) for hg in range(HG)]
    wvTs = [np.ascontiguousarray(Wv[hg * CH:(hg + 1) * CH, :].T).astype(bf) for hg in range(HG)]
    wpTs = [np.ascontiguousarray(Wp[:, hg * CH:(hg + 1) * CH].T).astype(bf) for hg in range(HG)]
    for core in range(8):
        b, hg = core // HG, core % HG
        in_maps.append({
            "xqT": xqT[b], "xkT": xkT[b], "xvT": xvT[b],
            "wqT": wqTs[hg], "wkT": wkTs[hg], "wvT": wvTs[hg],
            "wpT": wpTs[hg],
        })
    return in_maps


def kernel(xq, xk, xv, Wq, Wk, Wv, Wp, bp):
    xq, xk, xv = (np.asarray(t, np.float32) for t in (xq, xk, xv))
    Wq, Wk, Wv, Wp, bp = (np.asarray(t, np.float32) for t in (Wq, Wk, Wv, Wp, bp))
    nc = get_nc()
    in_maps = _prep_in_maps(xq, xk, xv, Wq, Wk, Wv, Wp)
    res = run_bass_kernel_spmd(nc, in_maps, list(range(8)))

    x_out = np.empty((B, N, C), np.float32)
    attn = np.empty((B, N, N), np.float32)
    for b in range(B):
        r0, r1 = res.results[2 * b], res.results[2 * b + 1]
        attn[b] = (r0["attn_p"] + r1["attn_p"]) * (1.0 / H)
        x_out[b] = r0["y_p"] + r1["y_p"] + bp
    return x_out.swapaxes(0, 1), attn


# revision 19
# speedup vs baseline: 1.0707x; 1.0091x over previous
"""Multi-head attention (B=4, N=1024, C=1024, H=16) on 8 TRN2 NeuronCores.

Sharding: batch B (4-way) x head-group (2-way, 8 heads each).
Core i handles batch b=i//2, head-group hg=i%2.

Per-core outputs: attn_p [Nq,Nk] (sum of the group's 8 scaled score maps)
and y_p [N,C] (projection partial over the group's 512 channels).
Host unshard: pair-sum + bias + /16 head-mean + final transposes.
"""
import numpy as np
import ml_dtypes

import concourse.mybir as mybir
import concourse.tile as tile
from concourse import bacc
from concourse.bass_utils import run_bass_kernel_spmd

P = 128
B, N, C, H = 4, 1024, 1024, 16
HG = 2            # head-group shards
CH = C // HG      # 512 channels per core
NHEAD = H // HG   # 8 heads per core
D = C // H        # 64 head dim
KC = C // P       # 8 contraction chunks over C
CC = CH // P      # 4 chunks over the 512 sharded channels
QT = N // P       # 8 token tiles of 128
QH = N // 512     # 2 token halves of 512
BF = mybir.dt.bfloat16
F32 = mybir.dt.float32

_NC_CACHE = None


def _build_nc(loop_n=1, variant=""):
    vset = set(variant.split(",")) if variant else set()
    nc = bacc.Bacc("TRN2", target_bir_lowering=False, debug=False, num_devices=8)

    xqT = nc.dram_tensor("xqT", [C, N], BF, kind="ExternalInput")
    xkT = nc.dram_tensor("xkT", [C, N], BF, kind="ExternalInput")
    xvT = nc.dram_tensor("xvT", [C, N], BF, kind="ExternalInput")
    wqT = nc.dram_tensor("wqT", [C, CH], BF, kind="ExternalInput")
    wkT = nc.dram_tensor("wkT", [C, CH], BF, kind="ExternalInput")
    wvT = nc.dram_tensor("wvT", [C, CH], BF, kind="ExternalInput")
    wpT = nc.dram_tensor("wpT", [CH, C], BF, kind="ExternalInput")
    attn_p = nc.dram_tensor("attn_p", [N, N], F32, kind="ExternalOutput")
    y_p = nc.dram_tensor("y_p", [N, C], F32, kind="ExternalOutput")

    xqT3 = xqT.rearrange("(ko p) n -> p ko n", p=P)
    xkT3 = xkT.rearrange("(ko p) n -> p ko n", p=P)
    xvT3 = xvT.rearrange("(ko p) n -> p ko n", p=P)
    wqT3 = wqT.rearrange("(ko p) m -> p ko m", p=P)
    wkT3 = wkT.rearrange("(ko p) m -> p ko m", p=P)
    wvT3 = wvT.rearrange("(ko p) m -> p ko m", p=P)
    wpT3 = wpT.rearrange("(ko p) m -> p ko m", p=P)
    attn3 = attn_p.rearrange("(qo p) k -> p qo k", p=P)
    y3 = y_p.rearrange("(to p) c -> p to c", p=P)

    with tile.TileContext(nc) as tc:
        with (
            tc.tile_pool(name="inp", bufs=1) as inp,
            tc.tile_pool(name="w", bufs=1) as wp_,
            tc.tile_pool(name="qkv", bufs=1) as qkv,
            tc.tile_pool(name="pt", bufs=5) as ptp,
            tc.tile_pool(name="sm", bufs=2) as smp,
            tc.tile_pool(name="st", bufs=3) as stp,
            tc.tile_pool(name="psA", bufs=2, space="PSUM") as psA,
            tc.tile_pool(name="psS", bufs=3, space="PSUM") as psS,
            tc.tile_pool(name="psO", bufs=3, space="PSUM") as psO,
        ):
            if loop_n > 1 and "dma_out" not in vset:
                loop_ctx = tc.For_i(0, loop_n, 1)
                loop_ctx.__enter__()
            # ---- input / weight loads (chunked for DMA-queue parallelism) ----
            xq_sb = inp.tile([P, KC, N], BF, tag="xq")
            xk_sb = inp.tile([P, KC, N], BF, tag="xk")
            xv_sb = inp.tile([P, KC, N], BF, tag="xv")
            wq_sb = wp_.tile([P, KC, CH], BF, tag="wq")
            wk_sb = wp_.tile([P, KC, CH], BF, tag="wk")
            wv_sb = wp_.tile([P, KC, CH], BF, tag="wv")
            wpp_sb = wp_.tile([P, CC, C], BF, tag="wp")
            for kc in range(KC):
                nc.sync.dma_start(xq_sb[:, kc, :], xqT3[:, kc, :])
                nc.sync.dma_start(wq_sb[:, kc, :], wqT3[:, kc, :])
            for kc in range(KC):
                nc.sync.dma_start(xk_sb[:, kc, :], xkT3[:, kc, :])
                nc.sync.dma_start(wk_sb[:, kc, :], wkT3[:, kc, :])
            for kc in range(KC):
                nc.sync.dma_start(xv_sb[:, kc, :], xvT3[:, kc, :])
                nc.sync.dma_start(wv_sb[:, kc, :], wvT3[:, kc, :])
            for cc in range(CC):
                nc.sync.dma_start(wpp_sb[:, cc, :], wpT3[:, cc, :])

            if loop_n > 1 and "dma_out" in vset:
                loop_ctx = tc.For_i(0, loop_n, 1)
                loop_ctx.__enter__()
            qt_sb = qkv.tile([P, CC, N], BF, tag="qt")    # Q^T (pre-scaled by 1/8)
            kt_sb = qkv.tile([P, CC, N], BF, tag="kt")    # K^T
            v65_sb = qkv.tile([P, QT, NHEAD * (D + 1)], BF, tag="v65")
            ot_sb = qkv.tile([P, CC, N], BF, tag="ot")    # O^T (normalized)

            def gen_qk(cc):
                """Q^T and K^T chunk cc: 32 matmul quanta."""
                for w_sb, x_sb, o_sb, scale in (
                    (wq_sb, xq_sb, qt_sb, 0.125),
                    (wk_sb, xk_sb, kt_sb, None),
                ):
                    for qh in range(QH):
                        pq = psA.tile([P, 512], F32, tag="pa", name="psqk")
                        for kc in range(KC):
                            nc.tensor.matmul(
                                pq[:],
                                lhsT=w_sb[:, kc, cc * P:(cc + 1) * P],
                                rhs=x_sb[:, kc, qh * 512:(qh + 1) * 512],
                                start=(kc == 0),
                                stop=(kc == KC - 1),
                            )
                            yield
                        dst = o_sb[:, cc, qh * 512:(qh + 1) * 512]
                        if scale is not None:
                            nc.vector.tensor_scalar_mul(dst, pq[:], scale)
                        else:
                            nc.vector.tensor_copy(out=dst, in_=pq[:])
                yield f"qk{cc}"

            def gen_v():
                """V projection into [tok, head*(64+1)] layout: 64 quanta."""
                for tt in range(QT):
                    pv = psA.tile([P, 512], F32, tag="pa", name="pv")
                    for kc in range(KC):
                        nc.tensor.matmul(
                            pv[:],
                            lhsT=xv_sb[:, kc, tt * P:(tt + 1) * P],
                            rhs=wv_sb[:, kc, :],
                            start=(kc == 0),
                            stop=(kc == KC - 1),
                        )
                        yield
                    for h in range(NHEAD):
                        nc.vector.tensor_copy(
                            out=v65_sb[:, tt, h * (D + 1): h * (D + 1) + D],
                            in_=pv[:, h * D:(h + 1) * D],
                        )
                    nc.any.memset(v65_sb[:, tt, D::(D + 1)], 1.0)

            pts_by_pair = {}

            def gen_scores(hp):
                """16 slots; each: 2 row-packed matmuls (concurrent) + 2 exps."""
                pts = [ptp.tile([P, QT, N], BF, tag="pt", name=f"pt{h2}")
                       for h2 in range(2)]
                pts_by_pair[hp] = pts
                for kt in range(QT):
                    for qh in range(QH):
                        pss = []
                        for h2 in range(2):
                            base = 64 * h2
                            ps = psS.tile([P, 512], F32, tag="ps", name="ps")
                            nc.tensor.matmul(
                                ps[:],
                                lhsT=kt_sb[base:base + 64, hp, kt * P:(kt + 1) * P],
                                rhs=qt_sb[base:base + 64, hp, qh * 512:(qh + 1) * 512],
                                start=True,
                                stop=True,
                            )
                            pss.append(ps)
                        if "noexp" not in vset:
                            for h2 in range(2):
                                nc.scalar.activation(
                                    pts[h2][:, kt, qh * 512:(qh + 1) * 512], pss[h2][:],
                                    mybir.ActivationFunctionType.Exp,
                                )
                        yield

            def gen_pv(hp):
                """PV + normalize for head pair hp: 32 quanta."""
                pts = pts_by_pair[hp]
                for h2 in range(2):
                    h = 2 * hp + h2
                    base = 64 * h2
                    pos = [psO.tile([65, 512], F32, tag="po", name=f"po{qh}")
                           for qh in range(QH)]
                    for kt in range(QT):
                        for qh in range(QH):
                            nc.tensor.matmul(
                                pos[qh][:],
                                lhsT=v65_sb[:, kt, h * (D + 1):(h + 1) * (D + 1)],
                                rhs=pts[h2][:, kt, qh * 512:(qh + 1) * 512],
                                start=(kt == 0),
                                stop=(kt == QT - 1),
                            )
                            yield
                    for qh in range(QH if "nonorm" not in vset else 0):
                        recip = smp.tile([1, 512], F32, tag="recip", name="recip")
                        nc.vector.reciprocal(recip[:], pos[qh][64:65, :])
                        bc = smp.tile([64, 512], F32, tag="bc", name="bc")
                        nc.gpsimd.partition_broadcast(bc[:], recip[:])
                        nc.vector.tensor_mul(
                            out=ot_sb[base:base + 64, hp, qh * 512:(qh + 1) * 512],
                            in0=pos[qh][0:64, :],
                            in1=bc[:],
                        )

            def gen_b():
                """attn partial (full 512-ch contraction): 64 quanta."""
                for qt in range(QT):
                    for kh in range(QH):
                        pb = psA.tile([P, 512], F32, tag="pa", name="pat")
                        for cc in range(CC):
                            nc.tensor.matmul(
                                pb[:],
                                lhsT=qt_sb[:, cc, qt * P:(qt + 1) * P],
                                rhs=kt_sb[:, cc, kh * 512:(kh + 1) * 512],
                                start=(cc == 0),
                                stop=(cc == CC - 1),
                            )
                            yield
                        stg = stp.tile([P, 512], F32, tag="stg", name="stga")
                        nc.vector.tensor_copy(out=stg[:], in_=pb[:])
                        nc.sync.dma_start(attn3[:, qt, kh * 512:(kh + 1) * 512], stg[:])

            def gen_d():
                """output projection y partial [N, C]: 64 quanta."""
                for qt in range(QT):
                    for ch2 in range(QH):
                        pd = psA.tile([P, 512], F32, tag="pa", name="py")
                        for cc in range(CC):
                            nc.tensor.matmul(
                                pd[:],
                                lhsT=ot_sb[:, cc, qt * P:(qt + 1) * P],
                                rhs=wpp_sb[:, cc, ch2 * 512:(ch2 + 1) * 512],
                                start=(cc == 0),
                                stop=(cc == CC - 1),
                            )
                            yield
                        stg = stp.tile([P, 512], F32, tag="stg", name="stgy")
                        nc.vector.tensor_copy(out=stg[:], in_=pd[:])
                        nc.sync.dma_start(y3[:, qt, ch2 * 512:(ch2 + 1) * 512], stg[:])

            # interleaved emission: qk0 first, then 1 score slot : 4 other MMs.
            # scores for pair hp are gated on the "qk{hp}" marker so every
            # kt/qt evacuation is emitted before its first reader.
            import itertools
            for _ in gen_qk(0):
                pass
            _trunc = 5
            for v in vset:
                if v.startswith("t") and v[1:].isdigit():
                    _trunc = int(v[1:])
            _chain = [gen_qk(1), gen_v(), gen_qk(2), gen_qk(3)]
            if _trunc >= 3:
                _chain += [gen_pv(0), gen_pv(1)]
            if _trunc >= 4:
                _chain.append(gen_b())
            if _trunc >= 3:
                _chain += [gen_pv(2), gen_pv(3)]
            if _trunc >= 5:
                _chain.append(gen_d())
            others = itertools.chain(*_chain)
            _SENT = object()
            events = {"qk0"}

            def pump():
                x = next(others, _SENT)
                if x is _SENT:
                    return False
                if isinstance(x, str):
                    events.add(x)
                return True

            trunc = 5
            for v in vset:
                if v.startswith("t") and v[1:].isdigit():
                    trunc = int(v[1:])
            if trunc <= 1:
                # phase A only: qk chunks + V
                for g in (gen_qk(1), gen_v(), gen_qk(2), gen_qk(3)):
                    for _ in g:
                        pass
            else:
                for hp in range(NHEAD // 2):
                    while f"qk{hp}" not in events:
                        if not pump():
                            break
                    for _slot in gen_scores(hp):
                        for _i in range(4):
                            pump()
                while pump():
                    pass

            if loop_n > 1:
                loop_ctx.__exit__(None, None, None)

    nc.compile()
    return nc


def get_nc():
    global _NC_CACHE
    if _NC_CACHE is None:
        _NC_CACHE = _build_nc()
    return _NC_CACHE


def _prep_in_maps(xq, xk, xv, Wq, Wk, Wv, Wp):
    bf = ml_dtypes.bfloat16
    in_maps = []
    xqT = [np.ascontiguousarray(xq[b].T).astype(bf) for b in range(B)]
    xkT = [np.ascontiguousarray(xk[b].T).astype(bf) for b in range(B)]
    xvT = [np.ascontiguousarray(xv[b].T).astype(bf) for b in range(B)]
    wqTs = [np.ascontiguousarray(Wq[hg * CH:(hg + 1) * CH, :].T).astype(bf) for hg in range(HG)]
    wkTs = [np.ascontiguousarray(Wk[hg * CH:(hg + 1) * CH, :].T).astype(bf) for hg in range(HG)]
    wvTs = [np.ascontiguousarray(Wv[hg * CH:(hg + 1) * CH, :].T).astype(bf) for hg in range(HG)]
    wpTs = [np.ascontiguousarray(Wp[:, hg * CH:(hg + 1) * CH].T).astype(bf) for hg in range(HG)]
    for core in range(8):
        b, hg = core // HG, core % HG
        in_maps.append({
            "xqT": xqT[b], "xkT": xkT[b], "xvT": xvT[b],
            "wqT": wqTs[hg], "wkT": wkTs[hg], "wvT": wvTs[hg],
            "wpT": wpTs[hg],
        })
    return in_maps


def kernel(xq, xk, xv, Wq, Wk, Wv, Wp, bp):
    xq, xk, xv = (np.asarray(t, np.float32) for t in (xq, xk, xv))
    Wq, Wk, Wv, Wp, bp = (np.asarray(t, np.float32) for t in (Wq, Wk, Wv, Wp, bp))
    nc = get_nc()
    in_maps = _prep_in_maps(xq, xk, xv, Wq, Wk, Wv, Wp)
    res = run_bass_kernel_spmd(nc, in_maps, list(range(8)))

    x_out = np.empty((B, N, C), np.float32)
    attn = np.empty((B, N, N), np.float32)
    for b in range(B):
        r0, r1 = res.results[2 * b], res.results[2 * b + 1]
        attn[b] = (r0["attn_p"] + r1["attn_p"]) * (1.0 / H)
        x_out[b] = r0["y_p"] + r1["y_p"] + bp
    return x_out.swapaxes(0, 1), attn
